# revision 3
# baseline (speedup 1.0000x reference)
"""COSNCE loss kernel for 8 Trainium2 NeuronCores.

Math: the reference loss collapses to two scalar reductions.
  With x_hat = l2norm(r_s[i,s,:]), y_hat = l2norm(r_t[i,s,:]):
    S_diag = sum_{i,s} <x_hat[i,s], y_hat[i,s]>
    S_all  = sum_{i,j,s} <x_hat[i,s], y_hat[j,s]>
           = sum_s <A_s, B_s>,  A_s = sum_i x_hat[i,s], B_s = sum_j y_hat[j,s]
  loss = (2*b*seq - S_diag*(1 + (K+1)/(2K)) + S_all/(2K)) / (b*200*(K+1)*4)
  (K = b-1).  So no [b,b,seq] einsum is needed - the kernel is a pure
  memory-bound streaming reduction over r_s and r_t.

Distribution: data-parallel over seq (2048 -> 8 x 256).  Every term above is
local to a seq position, so each core computes partial (S_diag, S_all) for its
seq-shard and the host sums 8 tiny vectors.  No device collectives.

Inputs are downcast to bf16 on the host: the loss numerator is dominated by a
constant (2*b*seq = 262144) and the data-dependent part is ~1e-4 relative, so
bf16 quantization error lands around 1e-6 relative on the output while halving
both HBM traffic and vector-engine cycles.
"""

import os
import sys

sys.path.insert(0, "/opt/trn_rl_repo")

import ml_dtypes
import numpy as np

B, SEQ, HID = 64, 2048, 256
NCORES = 8
S_SHARD = SEQ // NCORES          # 256 seq positions per core
P = 128                          # SBUF partitions
N_SSUB = S_SHARD // P            # 2 partition-blocks of seq per core
G = 8                            # i-blocks per group (batched rsqrt)
K = B - 1

LAST_RESULTS = None              # BassKernelResults of the last run (for test.py)


def _build_program():
    import concourse.bacc as bacc
    import concourse.mybir as mybir
    import concourse.tile as tile

    f32 = mybir.dt.float32
    bf16 = mybir.dt.bfloat16
    mult = mybir.AluOpType.mult
    add = mybir.AluOpType.add
    AX = mybir.AxisListType.X

    nc = bacc.Bacc("TRN2", num_devices=NCORES)
    x_dram = nc.dram_tensor("r_s", [B, S_SHARD, HID], bf16, kind="ExternalInput")
    y_dram = nc.dram_tensor("r_t", [B, S_SHARD, HID], bf16, kind="ExternalInput")
    # out columns: (diag_ssub0, ab_ssub0, diag_ssub1, ab_ssub1)
    out_dram = nc.dram_tensor("out", [P, 2 * N_SSUB], f32, kind="ExternalOutput")

    with tile.TileContext(nc) as tc:
        with (
            tc.tile_pool(name="persist", bufs=1) as persist,
            tc.tile_pool(name="xin", bufs=18) as xpool,
            tc.tile_pool(name="yin", bufs=18) as ypool,
            tc.tile_pool(name="scr", bufs=8) as spool,
            tc.tile_pool(name="small", bufs=4) as smallp,
        ):
            out_sb = persist.tile([P, 2 * N_SSUB], f32, tag="out_sb")

            for ssub in range(N_SSUB):
                s0 = ssub * P
                A = persist.tile([P, HID], bf16, tag=f"A{ssub}")
                Bacc = persist.tile([P, HID], bf16, tag=f"B{ssub}")
                ssqx = persist.tile([P, B], f32, tag=f"ssqx{ssub}")
                ssqy = persist.tile([P, B], f32, tag=f"ssqy{ssub}")
                dotb = persist.tile([P, B], f32, tag=f"dotb{ssub}")
                invx = persist.tile([P, B], f32, tag=f"invx{ssub}")
                invy = persist.tile([P, B], f32, tag=f"invy{ssub}")

                for g in range(B // G):
                    xts = []
                    yts = []
                    for k in range(G):
                        i = g * G + k
                        xt = xpool.tile([P, HID], bf16, tag="xt")
                        yt = ypool.tile([P, HID], bf16, tag="yt")
                        nc.sync.dma_start(out=xt[:], in_=x_dram[i, s0 : s0 + P, :])
                        nc.sync.dma_start(out=yt[:], in_=y_dram[i, s0 : s0 + P, :])
                        xts.append(xt)
                        yts.append(yt)
                        # ssq_x[i], ssq_y[i], dot[i] via fused multiply+reduce
                        # fused square/dot + free-dim reduce in one DVE op:
                        #   out = (in0 * 1) * in1 ; accum_out = sum(out)
                        scr = spool.tile([P, HID], bf16, tag="scr")
                        nc.vector.scalar_tensor_tensor(
                            out=scr[:], in0=xt[:], scalar=1.0, in1=xt[:],
                            op0=mult, op1=mult, accum_out=ssqx[:, i : i + 1],
                        )
                        scr = spool.tile([P, HID], bf16, tag="scr")
                        nc.vector.scalar_tensor_tensor(
                            out=scr[:], in0=yt[:], scalar=1.0, in1=yt[:],
                            op0=mult, op1=mult, accum_out=ssqy[:, i : i + 1],
                        )
                        scr = spool.tile([P, HID], bf16, tag="scr")
                        nc.vector.scalar_tensor_tensor(
                            out=scr[:], in0=xt[:], scalar=1.0, in1=yt[:],
                            op0=mult, op1=mult, accum_out=dotb[:, i : i + 1],
                        )

                    gsl = slice(g * G, (g + 1) * G)
                    # inv_norm = sqrt(1/ssq); ACT Rsqrt is banned for accuracy,
                    # DVE reciprocal + ACT sqrt is the sanctioned path.
                    rx = smallp.tile([P, G], f32, tag="rx")
                    nc.vector.reciprocal(rx[:], ssqx[:, gsl])
                    nc.scalar.sqrt(invx[:, gsl], rx[:])
                    ry = smallp.tile([P, G], f32, tag="ry")
                    nc.vector.reciprocal(ry[:], ssqy[:, gsl])
                    nc.scalar.sqrt(invy[:, gsl], ry[:])

                    for k in range(G):
                        i = g * G + k
                        if i == 0:
                            nc.vector.tensor_scalar(
                                out=A[:], in0=xts[k][:],
                                scalar1=invx[:, i : i + 1], scalar2=None, op0=mult,
                            )
                            nc.vector.tensor_scalar(
                                out=Bacc[:], in0=yts[k][:],
                                scalar1=invy[:, i : i + 1], scalar2=None, op0=mult,
                            )
                        else:
                            nc.vector.scalar_tensor_tensor(
                                out=A[:], in0=xts[k][:], scalar=invx[:, i : i + 1],
                                in1=A[:], op0=mult, op1=add,
                            )
                            nc.vector.scalar_tensor_tensor(
                                out=Bacc[:], in0=yts[k][:], scalar=invy[:, i : i + 1],
                                in1=Bacc[:], op0=mult, op1=add,
                            )

                # per-ssub epilogue: diag partial and <A,B> partial
                t1 = smallp.tile([P, B], f32, tag="t1")
                nc.vector.tensor_tensor(out=t1[:], in0=dotb[:], in1=invx[:], op=mult)
                t2 = smallp.tile([P, B], f32, tag="t2")
                nc.vector.tensor_tensor(out=t2[:], in0=t1[:], in1=invy[:], op=mult)
                nc.vector.reduce_sum(
                    out=out_sb[:, 2 * ssub : 2 * ssub + 1], in_=t2[:], axis=AX
                )
                scr = spool.tile([P, HID], bf16, tag="scr")
                nc.vector.scalar_tensor_tensor(
                    out=scr[:], in0=A[:], scalar=1.0, in1=Bacc[:],
                    op0=mult, op1=mult,
                    accum_out=out_sb[:, 2 * ssub + 1 : 2 * ssub + 2],
                )

            nc.sync.dma_start(out=out_dram[:, :], in_=out_sb[:])

    nc.compile()
    return nc


_PROGRAM = None


def kernel(r_s: np.ndarray, r_t: np.ndarray) -> np.ndarray:
    global _PROGRAM, LAST_RESULTS
    from concourse.bass_utils import run_bass_kernel_spmd

    r_s = np.asarray(r_s)
    r_t = np.asarray(r_t)
    assert r_s.shape == (B, SEQ, HID) and r_t.shape == (B, SEQ, HID)

    rs16 = r_s.astype(ml_dtypes.bfloat16)
    rt16 = r_t.astype(ml_dtypes.bfloat16)
    in_maps = []
    for c in range(NCORES):
        sl = slice(c * S_SHARD, (c + 1) * S_SHARD)
        in_maps.append(
            {
                "r_s": np.ascontiguousarray(rs16[:, sl, :]),
                "r_t": np.ascontiguousarray(rt16[:, sl, :]),
            }
        )

    if _PROGRAM is None:
        _PROGRAM = _build_program()

    res = run_bass_kernel_spmd(
        _PROGRAM,
        in_maps,
        core_ids=list(range(NCORES)),
        trace=bool(int(os.environ.get("COSNCE_TRACE", "0"))),
    )
    LAST_RESULTS = res

    # host reduction of the tiny per-core partials (float64 for stability)
    s_diag = 0.0
    s_all = 0.0
    for r in res.results:
        o = r["out"].astype(np.float64)
        s_diag += o[:, 0::2].sum()
        s_all += o[:, 1::2].sum()

    kf = float(K)
    numer = 2.0 * B * SEQ - s_diag * (1.0 + (kf + 1.0) / (2.0 * kf)) + s_all / (2.0 * kf)
    loss = numer / (B * 200 * (K + 1) * 4)
    return np.float32(loss)


# revision 4
# speedup vs baseline: 1.4786x; 1.4786x over previous
"""COSNCE loss kernel for 8 Trainium2 NeuronCores.

Math: the reference loss collapses to two scalar reductions.
  With x_hat = l2norm(r_s[i,s,:]), y_hat = l2norm(r_t[i,s,:]):
    S_diag = sum_{i,s} <x_hat[i,s], y_hat[i,s]>
    S_all  = sum_{i,j,s} <x_hat[i,s], y_hat[j,s]>
           = sum_s <A_s, B_s>,  A_s = sum_i x_hat[i,s], B_s = sum_j y_hat[j,s]
  loss = (2*b*seq - S_diag*(1 + (K+1)/(2K)) + S_all/(2K)) / (b*200*(K+1)*4)
  (K = b-1).  So no [b,b,seq] einsum is needed - the kernel is a pure
  memory-bound streaming reduction over r_s and r_t.

Distribution: data-parallel over seq (2048 -> 8 x 256).  Every term above is
local to a seq position, so each core computes partial (S_diag, S_all) for its
seq-shard and the host sums 8 tiny vectors.  No device collectives.

Inputs are downcast to bf16 on the host: the loss numerator is dominated by a
constant (2*b*seq = 262144) and the data-dependent part is ~1e-4 relative, so
bf16 quantization error lands around 1e-6 relative on the output while halving
both HBM traffic and vector-engine cycles.

Engine split (per [128 seq, 256 hid] block, layout partition=seq, free=hid):
  - DVE: ssq_x and dot via 2x-rate tensor_tensor + segmented reduce (batched
    over 8 blocks); diag(inv) builds via 4x-rate tensor_scalar on an identity.
  - ACT: ssq_y via Square activation with accum_out; sqrt of 1/ssq.
  - PE:  the normalize-and-accumulate A += diag(inv_x) @ X as accumulating
    matmuls into PSUM (f32), replacing per-block DVE scalar_tensor_tensor.
"""

import os
import sys

sys.path.insert(0, "/opt/trn_rl_repo")

import ml_dtypes
import numpy as np

B, SEQ, HID = 64, 2048, 256
NCORES = 8
S_SHARD = SEQ // NCORES          # 256 seq positions per core
P = 128                          # SBUF partitions
N_SSUB = S_SHARD // P            # 2 partition-blocks of seq per core
G = 8                            # i-blocks per group (batched ops + rsqrt)
K = B - 1

LAST_RESULTS = None              # BassKernelResults of the last run (for test.py)


def _build_program():
    import concourse.bacc as bacc
    import concourse.bass as bass
    import concourse.mybir as mybir
    import concourse.tile as tile

    f32 = mybir.dt.float32
    bf16 = mybir.dt.bfloat16
    mult = mybir.AluOpType.mult
    add = mybir.AluOpType.add
    AX = mybir.AxisListType.X
    Square = mybir.ActivationFunctionType.Square

    nc = bacc.Bacc("TRN2", num_devices=NCORES)
    x_dram = nc.dram_tensor("r_s", [B, S_SHARD, HID], bf16, kind="ExternalInput")
    y_dram = nc.dram_tensor("r_t", [B, S_SHARD, HID], bf16, kind="ExternalInput")
    eye_dram = nc.dram_tensor("eye", [P, P], bf16, kind="ExternalInput")
    # out columns: (diag_ssub0, ab_ssub0, diag_ssub1, ab_ssub1)
    out_dram = nc.dram_tensor("out", [P, 2 * N_SSUB], f32, kind="ExternalOutput")

    with tile.TileContext(nc) as tc:
        with (
            tc.tile_pool(name="persist", bufs=1) as persist,
            tc.tile_pool(name="gin", bufs=3) as gpool,
            tc.tile_pool(name="scr", bufs=3) as spool,
            tc.tile_pool(name="ascr", bufs=4) as apool,
            tc.tile_pool(name="diag", bufs=6) as dpool,
            tc.tile_pool(name="small", bufs=4) as smallp,
            tc.tile_pool(name="psum", bufs=1, space="PSUM") as pspool,
        ):
            out_sb = persist.tile([P, 2 * N_SSUB], f32, tag="out_sb")
            eye_t = persist.tile([P, P], bf16, tag="eye")
            nc.sync.dma_start(out=eye_t[:], in_=eye_dram[:, :])

            for ssub in range(N_SSUB):
                s0 = ssub * P
                A_ps = pspool.tile([P, HID], f32, tag=f"A{ssub}")
                B_ps = pspool.tile([P, HID], f32, tag=f"B{ssub}")
                ssqx = persist.tile([P, B], f32, tag=f"ssqx{ssub}")
                ssqy = persist.tile([P, B], f32, tag=f"ssqy{ssub}")
                dotb = persist.tile([P, B], f32, tag=f"dotb{ssub}")
                invx = persist.tile([P, B], f32, tag=f"invx{ssub}")
                invy = persist.tile([P, B], f32, tag=f"invy{ssub}")

                for g in range(B // G):
                    g0 = g * G
                    gsl = slice(g0, g0 + G)
                    Xg = gpool.tile([P, G, HID], bf16, tag="xg")
                    Yg = gpool.tile([P, G, HID], bf16, tag="yg")
                    nc.sync.dma_start(
                        out=Xg[:],
                        in_=x_dram[g0 : g0 + G, s0 : s0 + P, :].rearrange(
                            "a p h -> p a h"
                        ),
                    )
                    nc.sync.dma_start(
                        out=Yg[:],
                        in_=y_dram[g0 : g0 + G, s0 : s0 + P, :].rearrange(
                            "a p h -> p a h"
                        ),
                    )

                    # ssq_x and dot: batched 2x TT + segmented 1x reduce (DVE)
                    sqx = spool.tile([P, G, HID], bf16, tag="sqx")
                    nc.vector.tensor_tensor(out=sqx[:], in0=Xg[:], in1=Xg[:], op=mult)
                    nc.vector.reduce_sum(out=ssqx[:, gsl], in_=sqx[:], axis=AX)
                    pxy = spool.tile([P, G, HID], bf16, tag="pxy")
                    nc.vector.tensor_tensor(out=pxy[:], in0=Xg[:], in1=Yg[:], op=mult)
                    nc.vector.reduce_sum(out=dotb[:, gsl], in_=pxy[:], axis=AX)

                    # ssq_y: ACT Square with per-partition accumulate
                    for k in range(G):
                        i = g0 + k
                        yscr = apool.tile([P, HID], bf16, tag="yscr")
                        nc.scalar.activation(
                            out=yscr[:], in_=Yg[:, k, :], func=Square,
                            accum_out=ssqy[:, i : i + 1],
                        )

                    # inv_norm = sqrt(1/ssq); ACT Rsqrt is banned for accuracy,
                    # DVE reciprocal + ACT sqrt is the sanctioned path.
                    rx = smallp.tile([P, G], f32, tag="rx")
                    nc.vector.reciprocal(rx[:], ssqx[:, gsl])
                    nc.scalar.sqrt(invx[:, gsl], rx[:])
                    ry = smallp.tile([P, G], f32, tag="ry")
                    nc.vector.reciprocal(ry[:], ssqy[:, gsl])
                    nc.scalar.sqrt(invy[:, gsl], ry[:])

                    # A += diag(inv_x) @ X on the tensor engine (PSUM f32 accum)
                    for k in range(G):
                        i = g0 + k
                        dgx = dpool.tile([P, P], bf16, tag="dgx")
                        nc.vector.tensor_scalar(
                            out=dgx[:], in0=eye_t[:],
                            scalar1=invx[:, i : i + 1], scalar2=None, op0=mult,
                        )
                        nc.tensor.matmul(
                            A_ps[:], dgx[:], Xg[:, k, :],
                            start=(i == 0), stop=(i == B - 1),
                        )
                        dgy = dpool.tile([P, P], bf16, tag="dgy")
                        nc.vector.tensor_scalar(
                            out=dgy[:], in0=eye_t[:],
                            scalar1=invy[:, i : i + 1], scalar2=None, op0=mult,
                        )
                        nc.tensor.matmul(
                            B_ps[:], dgy[:], Yg[:, k, :],
                            start=(i == 0), stop=(i == B - 1),
                        )

                # per-ssub epilogue: diag partial and <A,B> partial
                t1 = smallp.tile([P, B], f32, tag="t1")
                nc.vector.tensor_tensor(out=t1[:], in0=dotb[:], in1=invx[:], op=mult)
                t2 = smallp.tile([P, B], f32, tag="t2")
                nc.vector.tensor_tensor(out=t2[:], in0=t1[:], in1=invy[:], op=mult)
                nc.vector.reduce_sum(
                    out=out_sb[:, 2 * ssub : 2 * ssub + 1], in_=t2[:], axis=AX
                )
                Acp = apool.tile([P, HID], bf16, tag="acp")
                nc.vector.tensor_copy(Acp[:], A_ps[:])
                Bcp = apool.tile([P, HID], bf16, tag="bcp")
                nc.vector.tensor_copy(Bcp[:], B_ps[:])
                scr = apool.tile([P, HID], bf16, tag="abscr")
                nc.vector.scalar_tensor_tensor(
                    out=scr[:], in0=Acp[:], scalar=1.0, in1=Bcp[:],
                    op0=mult, op1=mult,
                    accum_out=out_sb[:, 2 * ssub + 1 : 2 * ssub + 2],
                )

            nc.sync.dma_start(out=out_dram[:, :], in_=out_sb[:])

    nc.compile()
    return nc


_PROGRAM = None
_EYE = None


def kernel(r_s: np.ndarray, r_t: np.ndarray) -> np.ndarray:
    global _PROGRAM, _EYE, LAST_RESULTS
    from concourse.bass_utils import run_bass_kernel_spmd

    r_s = np.asarray(r_s)
    r_t = np.asarray(r_t)
    assert r_s.shape == (B, SEQ, HID) and r_t.shape == (B, SEQ, HID)

    rs16 = r_s.astype(ml_dtypes.bfloat16)
    rt16 = r_t.astype(ml_dtypes.bfloat16)
    if _EYE is None:
        _EYE = np.eye(P, dtype=ml_dtypes.bfloat16)
    in_maps = []
    for c in range(NCORES):
        sl = slice(c * S_SHARD, (c + 1) * S_SHARD)
        in_maps.append(
            {
                "r_s": np.ascontiguousarray(rs16[:, sl, :]),
                "r_t": np.ascontiguousarray(rt16[:, sl, :]),
                "eye": _EYE,
            }
        )

    if _PROGRAM is None:
        _PROGRAM = _build_program()

    res = run_bass_kernel_spmd(
        _PROGRAM,
        in_maps,
        core_ids=list(range(NCORES)),
        trace=bool(int(os.environ.get("COSNCE_TRACE", "0"))),
    )
    LAST_RESULTS = res

    # host reduction of the tiny per-core partials (float64 for stability)
    s_diag = 0.0
    s_all = 0.0
    for r in res.results:
        o = r["out"].astype(np.float64)
        s_diag += o[:, 0::2].sum()
        s_all += o[:, 1::2].sum()

    kf = float(K)
    numer = 2.0 * B * SEQ - s_diag * (1.0 + (kf + 1.0) / (2.0 * kf)) + s_all / (2.0 * kf)
    loss = numer / (B * 200 * (K + 1) * 4)
    return np.float32(loss)


# revision 5
# speedup vs baseline: 1.7189x; 1.1625x over previous
"""COSNCE loss kernel for 8 Trainium2 NeuronCores.

Math: the reference loss collapses to two scalar reductions.
  With x_hat = l2norm(r_s[i,s,:]), y_hat = l2norm(r_t[i,s,:]):
    S_diag = sum_{i,s} <x_hat[i,s], y_hat[i,s]>
    S_all  = sum_{i,j,s} <x_hat[i,s], y_hat[j,s]>
           = sum_s <A_s, B_s>,  A_s = sum_i x_hat[i,s], B_s = sum_j y_hat[j,s]
  loss = (2*b*seq - S_diag*(1 + (K+1)/(2K)) + S_all/(2K)) / (b*200*(K+1)*4)
  (K = b-1).  So no [b,b,seq] einsum is needed - the kernel is a pure
  memory-bound streaming reduction over r_s and r_t.

Distribution: data-parallel over seq (2048 -> 8 x 256).  Every term above is
local to a seq position, so each core computes partial (S_diag, S_all) for its
seq-shard and the host sums 8 tiny vectors.  No device collectives.

Inputs are downcast to bf16 on the host: the loss numerator is dominated by a
constant (2*b*seq = 262144) and the data-dependent part is ~1e-4 relative, so
bf16 quantization error lands around 1e-6 relative on the output while halving
both HBM traffic and vector-engine cycles.

Engine split (per [128 seq, 256 hid] block; layout partition=seq, free=hid):
  - PE (diag-matmul trick): per-seq scaling IS a matmul by a diagonal matrix,
    and PSUM accumulates the batch sum for free:
       A  += diag(invx_i) @ X_i          (the normalized batch-sum)
       B  += diag(invy_i) @ Y_i
       W  += diag(invx_i*invy_i) @ (X_i o Y_i)   (S_diag = sum W at the end)
    Dense back-to-back matmuls keep the PE out of its low p-state.
  - DVE: the X*X / X*Y products (2x-rate tensor_tensor, batched 8 blocks per
    op), the segmented ssq_x reduce, and the batched diag(inv) builds
    (broadcast-AP tensor_tensor against a resident identity).
  - ACT: ssq_y (and a tunable fraction of ssq_x groups) via Square activation
    with accum_out; sqrt() of the DVE reciprocal for inv-norms.
"""

import os
import sys

sys.path.insert(0, "/opt/trn_rl_repo")

import ml_dtypes
import numpy as np

B, SEQ, HID = 64, 2048, 256
NCORES = 8
S_SHARD = SEQ // NCORES          # 256 seq positions per core
P = 128                          # SBUF partitions
N_SSUB = S_SHARD // P            # 2 partition-blocks of seq per core
G = 8                            # i-blocks per group (batched ops + rsqrt)
NG = B // G                      # groups per ssub
K = B - 1

# fraction of groups whose ssq_x runs on ACT instead of DVE (load balance)
ACT_SSQX_EVERY = 4               # every 4th group -> f = 0.25

LAST_RESULTS = None              # BassKernelResults of the last run (for test.py)


def _build_program():
    import concourse.bacc as bacc
    import concourse.mybir as mybir
    import concourse.tile as tile

    f32 = mybir.dt.float32
    bf16 = mybir.dt.bfloat16
    mult = mybir.AluOpType.mult
    AX = mybir.AxisListType.X
    Square = mybir.ActivationFunctionType.Square

    nc = bacc.Bacc("TRN2", num_devices=NCORES)
    x_dram = nc.dram_tensor("r_s", [B, S_SHARD, HID], bf16, kind="ExternalInput")
    y_dram = nc.dram_tensor("r_t", [B, S_SHARD, HID], bf16, kind="ExternalInput")
    eye_dram = nc.dram_tensor("eye", [P, P], bf16, kind="ExternalInput")
    # out columns: (diag_ssub0, ab_ssub0, diag_ssub1, ab_ssub1)
    out_dram = nc.dram_tensor("out", [P, 2 * N_SSUB], f32, kind="ExternalOutput")

    with tile.TileContext(nc) as tc:
        with (
            tc.tile_pool(name="persist", bufs=1) as persist,
            tc.tile_pool(name="gin", bufs=3) as gpool,
            tc.tile_pool(name="scr", bufs=2) as spool,
            tc.tile_pool(name="pxyp", bufs=3) as pxypool,
            tc.tile_pool(name="ascr", bufs=4) as apool,
            tc.tile_pool(name="diag", bufs=2) as dpool,
            tc.tile_pool(name="small", bufs=6) as smallp,
            tc.tile_pool(name="psum", bufs=2, space="PSUM") as pspool,
        ):
            out_sb = persist.tile([P, 2 * N_SSUB], f32, tag="out_sb")
            eye_t = persist.tile([P, P], bf16, tag="eye")
            nc.sync.dma_start(out=eye_t[:], in_=eye_dram[:, :])
            eyeb = eye_t[:, None, :].broadcast_to([P, G, P])

            for ssub in range(N_SSUB):
                s0 = ssub * P
                A_ps = pspool.tile([P, HID], f32, tag="A")
                B_ps = pspool.tile([P, HID], f32, tag="B")
                W_ps = pspool.tile([P, HID], f32, tag="W")

                for g in range(NG):
                    g0 = g * G
                    Xg = gpool.tile([P, G, HID], bf16, tag="xg")
                    Yg = gpool.tile([P, G, HID], bf16, tag="yg")
                    nc.sync.dma_start(
                        out=Xg[:],
                        in_=x_dram[g0 : g0 + G, s0 : s0 + P, :].rearrange(
                            "a p h -> p a h"
                        ),
                    )
                    nc.sync.dma_start(
                        out=Yg[:],
                        in_=y_dram[g0 : g0 + G, s0 : s0 + P, :].rearrange(
                            "a p h -> p a h"
                        ),
                    )

                    ssqxg = smallp.tile([P, G], f32, tag="ssqx")
                    ssqyg = smallp.tile([P, G], f32, tag="ssqy")

                    # ssq_x: DVE batched TT+reduce, every Nth group on ACT
                    if g % ACT_SSQX_EVERY == ACT_SSQX_EVERY - 1:
                        for k in range(G):
                            xscr = apool.tile([P, HID], bf16, tag="xscr")
                            nc.scalar.activation(
                                out=xscr[:], in_=Xg[:, k, :], func=Square,
                                accum_out=ssqxg[:, k : k + 1],
                            )
                    else:
                        sqx = spool.tile([P, G, HID], bf16, tag="sqx")
                        nc.vector.tensor_tensor(
                            out=sqx[:], in0=Xg[:], in1=Xg[:], op=mult
                        )
                        nc.vector.reduce_sum(out=ssqxg[:], in_=sqx[:], axis=AX)

                    # ssq_y: ACT Square with per-partition accumulate
                    for k in range(G):
                        yscr = apool.tile([P, HID], bf16, tag="yscr")
                        nc.scalar.activation(
                            out=yscr[:], in_=Yg[:, k, :], func=Square,
                            accum_out=ssqyg[:, k : k + 1],
                        )

                    # raw elementwise products X o Y (consumed by the W matmuls)
                    pxy = pxypool.tile([P, G, HID], bf16, tag="pxy")
                    nc.vector.tensor_tensor(out=pxy[:], in0=Xg[:], in1=Yg[:], op=mult)

                    # inv_norm = sqrt(1/ssq); ACT Rsqrt is banned for accuracy,
                    # DVE reciprocal + ACT sqrt is the sanctioned path.
                    rx = smallp.tile([P, G], f32, tag="rx")
                    nc.vector.reciprocal(rx[:], ssqxg[:])
                    invxg = smallp.tile([P, G], f32, tag="invx")
                    nc.scalar.sqrt(invxg[:], rx[:])
                    ry = smallp.tile([P, G], f32, tag="ry")
                    nc.vector.reciprocal(ry[:], ssqyg[:])
                    invyg = smallp.tile([P, G], f32, tag="invy")
                    nc.scalar.sqrt(invyg[:], ry[:])
                    wg = smallp.tile([P, G], f32, tag="wg")
                    nc.vector.tensor_tensor(out=wg[:], in0=invxg[:], in1=invyg[:], op=mult)

                    # batched diag(inv) builds: eye broadcast * inv broadcast
                    dgx = dpool.tile([P, G, P], bf16, tag="dgx")
                    nc.vector.tensor_tensor(
                        out=dgx[:], in0=eyeb, in1=invxg[:].broadcast_to([P, G, P]),
                        op=mult,
                    )
                    dgy = dpool.tile([P, G, P], bf16, tag="dgy")
                    nc.vector.tensor_tensor(
                        out=dgy[:], in0=eyeb, in1=invyg[:].broadcast_to([P, G, P]),
                        op=mult,
                    )
                    dgw = dpool.tile([P, G, P], bf16, tag="dgw")
                    nc.vector.tensor_tensor(
                        out=dgw[:], in0=eyeb, in1=wg[:].broadcast_to([P, G, P]),
                        op=mult,
                    )

                    # accumulate A/B/W in PSUM on the tensor engine
                    first = g == 0
                    last = g == NG - 1
                    for k in range(G):
                        st = first and k == 0
                        sp = last and k == G - 1
                        nc.tensor.matmul(
                            A_ps[:], dgx[:, k, :], Xg[:, k, :], start=st, stop=sp
                        )
                        nc.tensor.matmul(
                            B_ps[:], dgy[:, k, :], Yg[:, k, :], start=st, stop=sp
                        )
                        nc.tensor.matmul(
                            W_ps[:], dgw[:, k, :], pxy[:, k, :], start=st, stop=sp
                        )

                # per-ssub epilogue: S_diag partial = sum(W), and <A,B> partial
                nc.vector.reduce_sum(
                    out=out_sb[:, 2 * ssub : 2 * ssub + 1], in_=W_ps[:], axis=AX
                )
                Acp = apool.tile([P, HID], bf16, tag="acp")
                nc.vector.tensor_copy(Acp[:], A_ps[:])
                Bcp = apool.tile([P, HID], bf16, tag="bcp")
                nc.vector.tensor_copy(Bcp[:], B_ps[:])
                scr = apool.tile([P, HID], bf16, tag="abscr")
                nc.vector.scalar_tensor_tensor(
                    out=scr[:], in0=Acp[:], scalar=1.0, in1=Bcp[:],
                    op0=mult, op1=mult,
                    accum_out=out_sb[:, 2 * ssub + 1 : 2 * ssub + 2],
                )

            nc.sync.dma_start(out=out_dram[:, :], in_=out_sb[:])

    nc.compile()
    return nc


_PROGRAM = None
_EYE = None


def kernel(r_s: np.ndarray, r_t: np.ndarray) -> np.ndarray:
    global _PROGRAM, _EYE, LAST_RESULTS
    from concourse.bass_utils import run_bass_kernel_spmd

    r_s = np.asarray(r_s)
    r_t = np.asarray(r_t)
    assert r_s.shape == (B, SEQ, HID) and r_t.shape == (B, SEQ, HID)

    rs16 = r_s.astype(ml_dtypes.bfloat16)
    rt16 = r_t.astype(ml_dtypes.bfloat16)
    if _EYE is None:
        _EYE = np.eye(P, dtype=ml_dtypes.bfloat16)
    in_maps = []
    for c in range(NCORES):
        sl = slice(c * S_SHARD, (c + 1) * S_SHARD)
        in_maps.append(
            {
                "r_s": np.ascontiguousarray(rs16[:, sl, :]),
                "r_t": np.ascontiguousarray(rt16[:, sl, :]),
                "eye": _EYE,
            }
        )

    if _PROGRAM is None:
        _PROGRAM = _build_program()

    res = run_bass_kernel_spmd(
        _PROGRAM,
        in_maps,
        core_ids=list(range(NCORES)),
        trace=bool(int(os.environ.get("COSNCE_TRACE", "0"))),
    )
    LAST_RESULTS = res

    # host reduction of the tiny per-core partials (float64 for stability)
    s_diag = 0.0
    s_all = 0.0
    for r in res.results:
        o = r["out"].astype(np.float64)
        s_diag += o[:, 0::2].sum()
        s_all += o[:, 1::2].sum()

    kf = float(K)
    numer = 2.0 * B * SEQ - s_diag * (1.0 + (kf + 1.0) / (2.0 * kf)) + s_all / (2.0 * kf)
    loss = numer / (B * 200 * (K + 1) * 4)
    return np.float32(loss)


# revision 13
# speedup vs baseline: 1.8544x; 1.0788x over previous
"""COSNCE loss kernel for 8 Trainium2 NeuronCores.

Math: the reference loss collapses to two scalar reductions.
  With x_hat = l2norm(r_s[i,s,:]), y_hat = l2norm(r_t[i,s,:]):
    S_diag = sum_{i,s} <x_hat[i,s], y_hat[i,s]>
    S_all  = sum_{i,j,s} <x_hat[i,s], y_hat[j,s]>
           = sum_s <A_s, B_s>,  A_s = sum_i x_hat[i,s], B_s = sum_j y_hat[j,s]
  loss = (2*b*seq - S_diag*(1 + (K+1)/(2K)) + S_all/(2K)) / (b*200*(K+1)*4)
  (K = b-1).  So no [b,b,seq] einsum is needed - the kernel is a pure
  memory-bound streaming reduction over r_s and r_t.

Distribution: data-parallel over seq (2048 -> 8 x 256).  Every term above is
local to a seq position, so each core computes partial (S_diag, S_all) for its
seq-shard and the host sums 8 tiny vectors.  No device collectives.

Inputs are downcast to bf16 on the host: the loss numerator is dominated by a
constant (2*b*seq = 262144) and the data-dependent part is ~1e-4 relative, so
bf16 quantization error lands around 1e-6 relative on the output while halving
both HBM traffic and vector-engine cycles.

Engine split (per [128 seq, 256 hid] block; layout partition=seq, free=hid):
  - PE (diag-matmul trick): per-seq scaling IS a matmul by a diagonal matrix,
    and PSUM accumulates the batch sum for free:
       A  += diag(invx_i) @ X_i          (the normalized batch-sum)
       B  += diag(invy_i) @ Y_i
       W  += diag(invx_i*invy_i) @ (X_i o Y_i)   (S_diag = sum W at the end)
    Dense back-to-back matmuls keep the PE out of its low p-state.
  - DVE: the X*X / X*Y products (2x-rate tensor_tensor, batched 8 blocks per
    op), the segmented ssq_x reduce, and the batched diag(inv) builds
    (broadcast-AP tensor_tensor against a resident identity).
  - ACT: ssq_y (and a tunable fraction of ssq_x groups) via Square activation
    with accum_out; sqrt() of the DVE reciprocal for inv-norms.
"""

import os
import sys

sys.path.insert(0, "/opt/trn_rl_repo")

import ml_dtypes
import numpy as np

B, SEQ, HID = 64, 2048, 256
NCORES = 8
S_SHARD = SEQ // NCORES          # 256 seq positions per core
P = 128                          # SBUF partitions
N_SSUB = S_SHARD // P            # 2 partition-blocks of seq per core
G = 8                            # i-blocks per group (batched ops + rsqrt)
NG = B // G                      # groups per ssub
K = B - 1

# fraction of groups whose ssq_x runs on ACT instead of DVE (load balance)
ACT_SSQX_EVERY = 3               # every 3rd group -> f = 1/3

LAST_RESULTS = None              # BassKernelResults of the last run (for test.py)


def _build_program():
    import concourse.bacc as bacc
    import concourse.mybir as mybir
    import concourse.tile as tile

    f32 = mybir.dt.float32
    bf16 = mybir.dt.bfloat16
    mult = mybir.AluOpType.mult
    add = mybir.AluOpType.add
    AX = mybir.AxisListType.X
    Square = mybir.ActivationFunctionType.Square

    nc = bacc.Bacc("TRN2", num_devices=NCORES)
    x_dram = nc.dram_tensor("r_s", [B, S_SHARD, HID], bf16, kind="ExternalInput")
    y_dram = nc.dram_tensor("r_t", [B, S_SHARD, HID], bf16, kind="ExternalInput")
    eye_dram = nc.dram_tensor("eye", [P, P], bf16, kind="ExternalInput")
    # out columns: (diag_ssub0, ab_ssub0, diag_ssub1, ab_ssub1)
    out_dram = nc.dram_tensor("out", [P, 2 * N_SSUB], f32, kind="ExternalOutput")

    with tile.TileContext(nc) as tc:
        with (
            tc.tile_pool(name="persist", bufs=1) as persist,
            tc.tile_pool(name="gin", bufs=4) as gpool,
            tc.tile_pool(name="scr", bufs=2) as spool,
            tc.tile_pool(name="pxyp", bufs=4) as pxypool,
            tc.tile_pool(name="ascr", bufs=6) as apool,
            tc.tile_pool(name="diag", bufs=3) as dpool,
            tc.tile_pool(name="small", bufs=8) as smallp,
            tc.tile_pool(name="psum", bufs=2, space="PSUM") as pspool,
        ):
            out_sb = persist.tile([P, 2 * N_SSUB], f32, tag="out_sb")
            eye_t = persist.tile([P, P], bf16, tag="eye")
            nc.sync.dma_start(out=eye_t[:], in_=eye_dram[:, :])
            eyeb = eye_t[:, None, :].broadcast_to([P, G, P])

            for ssub in range(N_SSUB):
                s0 = ssub * P
                A_ps = pspool.tile([P, HID], f32, tag="A")
                B_ps = pspool.tile([P, HID], f32, tag="B")
                W_ps = pspool.tile([P, HID], f32, tag="W")

                for g in range(NG):
                    g0 = g * G
                    Xg = gpool.tile([P, G, HID], bf16, tag="xg")
                    Yg = gpool.tile([P, G, HID], bf16, tag="yg")
                    nc.sync.dma_start(
                        out=Xg[:],
                        in_=x_dram[g0 : g0 + G, s0 : s0 + P, :].rearrange(
                            "a p h -> p a h"
                        ),
                    )
                    nc.sync.dma_start(
                        out=Yg[:],
                        in_=y_dram[g0 : g0 + G, s0 : s0 + P, :].rearrange(
                            "a p h -> p a h"
                        ),
                    )

                    ssqxy = smallp.tile([P, 2 * G], f32, tag="ssqxy")
                    ssqxg = ssqxy[:, 0:G]
                    ssqyg = ssqxy[:, G : 2 * G]

                    # ssq_x: DVE batched TT+reduce, every Nth group on ACT
                    if g % ACT_SSQX_EVERY == ACT_SSQX_EVERY - 1:
                        for k in range(G):
                            xscr = apool.tile([P, HID], bf16, tag="xscr")
                            nc.scalar.activation(
                                out=xscr[:], in_=Xg[:, k, :], func=Square,
                                accum_out=ssqxy[:, k : k + 1],
                            )
                    else:
                        # flattened 2D APs keep the DVE in 2x packed mode
                        sqx = spool.tile([P, G * HID], bf16, tag="sqx")
                        xf = Xg[:].rearrange("p a h -> p (a h)")
                        nc.vector.tensor_tensor(out=sqx[:], in0=xf, in1=xf, op=mult)
                        nc.vector.reduce_sum(
                            out=ssqxg,
                            in_=sqx[:].rearrange("p (a h) -> p a h", a=G),
                            axis=AX,
                        )

                    # ssq_y: ACT Square with per-partition accumulate
                    for k in range(G):
                        yscr = apool.tile([P, HID], bf16, tag="yscr")
                        nc.scalar.activation(
                            out=yscr[:], in_=Yg[:, k, :], func=Square,
                            accum_out=ssqxy[:, G + k : G + k + 1],
                        )

                    # raw elementwise products X o Y (consumed by the W matmuls)
                    pxy = pxypool.tile([P, G, HID], bf16, tag="pxy")
                    nc.vector.tensor_tensor(
                        out=pxy[:].rearrange("p a h -> p (a h)"),
                        in0=Xg[:].rearrange("p a h -> p (a h)"),
                        in1=Yg[:].rearrange("p a h -> p (a h)"),
                        op=mult,
                    )

                    # inv_norm = sqrt(1/ssq); ACT Rsqrt is banned for accuracy,
                    # DVE reciprocal + ACT sqrt is the sanctioned path.
                    # x and y chains batched into one recip + one sqrt per group.
                    rxy = smallp.tile([P, 2 * G], f32, tag="rxy")
                    nc.vector.reciprocal(rxy[:], ssqxy[:])
                    invxy = smallp.tile([P, 2 * G], f32, tag="invxy")
                    nc.scalar.sqrt(invxy[:], rxy[:])
                    invxg = invxy[:, 0:G]
                    invyg = invxy[:, G : 2 * G]
                    wg = smallp.tile([P, G], f32, tag="wg")
                    nc.vector.tensor_tensor(out=wg[:], in0=invxg, in1=invyg, op=mult)

                    # batched diag(inv) builds: eye broadcast * inv broadcast
                    dgx = dpool.tile([P, G, P], bf16, tag="dgx")
                    nc.vector.tensor_tensor(
                        out=dgx[:], in0=eyeb, in1=invxg.broadcast_to([P, G, P]),
                        op=mult,
                    )
                    dgy = dpool.tile([P, G, P], bf16, tag="dgy")
                    nc.vector.tensor_tensor(
                        out=dgy[:], in0=eyeb, in1=invyg.broadcast_to([P, G, P]),
                        op=mult,
                    )
                    dgw = dpool.tile([P, G, P], bf16, tag="dgw")
                    nc.vector.tensor_tensor(
                        out=dgw[:], in0=eyeb, in1=wg[:].broadcast_to([P, G, P]),
                        op=mult,
                    )

                    # accumulate A/B/W in PSUM on the tensor engine.
                    # All A-matmuls first: they only need dgx, so the PE can
                    # start while dgy/dgw are still being built (the engine
                    # queue is in-order; interleaving would head-of-line block).
                    first = g == 0
                    last = g == NG - 1
                    for k in range(G):
                        nc.tensor.matmul(
                            A_ps[:], dgx[:, k, :], Xg[:, k, :],
                            start=(first and k == 0), stop=(last and k == G - 1),
                        )
                    for k in range(G):
                        nc.tensor.matmul(
                            B_ps[:], dgy[:, k, :], Yg[:, k, :],
                            start=(first and k == 0), stop=(last and k == G - 1),
                        )
                    for k in range(G):
                        nc.tensor.matmul(
                            W_ps[:], dgw[:, k, :], pxy[:, k, :],
                            start=(first and k == 0), stop=(last and k == G - 1),
                        )

                # per-ssub epilogue: S_diag partial = sum(W), and <A,B> partial
                nc.vector.reduce_sum(
                    out=out_sb[:, 2 * ssub : 2 * ssub + 1], in_=W_ps[:], axis=AX
                )
                Acp = apool.tile([P, HID], bf16, tag="acp")
                nc.vector.tensor_copy(Acp[:], A_ps[:])
                Bcp = apool.tile([P, HID], bf16, tag="bcp")
                nc.vector.tensor_copy(Bcp[:], B_ps[:])
                scr = apool.tile([P, HID], bf16, tag="abscr")
                nc.vector.scalar_tensor_tensor(
                    out=scr[:], in0=Acp[:], scalar=1.0, in1=Bcp[:],
                    op0=mult, op1=mult,
                    accum_out=out_sb[:, 2 * ssub + 1 : 2 * ssub + 2],
                )

            nc.sync.dma_start(out=out_dram[:, :], in_=out_sb[:])

    nc.compile()
    return nc


_PROGRAM = None
_EYE = None


def kernel(r_s: np.ndarray, r_t: np.ndarray) -> np.ndarray:
    global _PROGRAM, _EYE, LAST_RESULTS
    from concourse.bass_utils import run_bass_kernel_spmd

    r_s = np.asarray(r_s)
    r_t = np.asarray(r_t)
    assert r_s.shape == (B, SEQ, HID) and r_t.shape == (B, SEQ, HID)

    rs16 = r_s.astype(ml_dtypes.bfloat16)
    rt16 = r_t.astype(ml_dtypes.bfloat16)
    if _EYE is None:
        _EYE = np.eye(P, dtype=ml_dtypes.bfloat16)
    in_maps = []
    for c in range(NCORES):
        sl = slice(c * S_SHARD, (c + 1) * S_SHARD)
        in_maps.append(
            {
                "r_s": np.ascontiguousarray(rs16[:, sl, :]),
                "r_t": np.ascontiguousarray(rt16[:, sl, :]),
                "eye": _EYE,
            }
        )

    if _PROGRAM is None:
        _PROGRAM = _build_program()

    res = run_bass_kernel_spmd(
        _PROGRAM,
        in_maps,
        core_ids=list(range(NCORES)),
        trace=bool(int(os.environ.get("COSNCE_TRACE", "0"))),
    )
    LAST_RESULTS = res

    # host reduction of the tiny per-core partials (float64 for stability)
    s_diag = 0.0
    s_all = 0.0
    for r in res.results:
        o = r["out"].astype(np.float64)
        s_diag += o[:, 0::2].sum()
        s_all += o[:, 1::2].sum()

    kf = float(K)
    numer = 2.0 * B * SEQ - s_diag * (1.0 + (kf + 1.0) / (2.0 * kf)) + s_all / (2.0 * kf)
    loss = numer / (B * 200 * (K + 1) * 4)
    return np.float32(loss)
